# revision 1
# baseline (speedup 1.0000x reference)
"""Maxwell viscoelastic model (linear recurrence scan) on 8 Trainium2 NeuronCores.

Math (per trajectory, T timesteps):
    a_n = 1 - k*dt_n              (k = E/eta = 2)
    b_n = k*dt_n*eps_n
    gamma_n = a_n*gamma_{n-1} + b_n,  gamma_0 = 0
    sigma_n = (E_inf + E)*eps_n - E*gamma_n = 2.5*eps_n - 2*gamma_n

Kernel strategy: shard the batch (4096 trajectories) across 8 cores (512
each) — the recurrence is independent per trajectory, so pure data
parallelism.  Per core, 4 tiles of [128 partitions x 4096 timesteps], cut
into CH time-chunks that stream through a software pipeline.  The
recurrence runs on the DVE tensor_tensor_scan instruction:
    state = (data0 * state) + data1   per partition, along the free dim.
We scan g_n = a_n*g_{n-1} + (-E*b_n) so g = -E*gamma directly, then
sigma = (eps * 2.5) + g in one scalar_tensor_tensor op.

Engine split (the DVE's second SBUF read port is the one shared with
GpSimd, so every 2-input DVE op with both operands in SBUF locks GpSimd
out; routing one operand of each DVE op through PSUM frees that port):
  SYNC   loads xt chunks (HWDGE qSPDynamicHW ring)
  ACT    a = 1 - k*dt  -> PSUM, and issues output stores (qActDynamicHW)
  POOL   bneg = (dt * -E*k) * eps -> SBUF   (runs concurrently with DVE)
  DVE    scan(a[PSUM], bneg[SBUF]) -> g[PSUM]; sigma(eps[SBUF], g[PSUM])

Raw bass (no TileContext): the Tile scheduler attaches semaphore waits
directly to instructions and overflows the tiny ISA sync-wait budgets
(S2S2D2_STT takes a single wait; the tail Drain takes few). With raw bass
every wait is a standalone instruction and the pipeline is explicit.
DMA completion uses one semaphore per (buffer slot, chunk): two DMAs on
one ring can complete out of order, so a shared counter cannot tell which
transfer finished.
"""

import numpy as np

import concourse.bass as bass
import concourse.mybir as mybir
from concourse.bass_utils import run_bass_kernel_spmd

E = 2.0
ETA = 1.0
E_INFTY = 0.5
K = E / ETA                  # 2.0
NEG_EK = -(E * K)            # -4.0: scan data1 scale so the scan outputs -E*gamma
SIG_EPS = E_INFTY + E        # 2.5

N_CORES = 8
P = 128                      # SBUF partitions
CH = 4                       # time chunks per tile
XT_BUFS = 3                  # xt ring depth


def build_nc(b_shard: int, t_len: int) -> bass.Bass:
    nc = bass.Bass()
    x = nc.dram_tensor("x", [b_shard, t_len, 2], mybir.dt.float32, kind="ExternalInput")
    y = nc.dram_tensor("y", [b_shard, t_len], mybir.dt.float32, kind="ExternalOutput")
    n_tiles = b_shard // P
    assert n_tiles * P == b_shard
    assert t_len % CH == 0
    L = t_len // CH

    xr = x.rearrange("(n p) t c -> n p t c", p=P)   # [n_tiles, 128, T, 2]
    yr = y.rearrange("(n p) t -> n p t", p=P)       # [n_tiles, 128, T]
    f32 = mybir.dt.float32
    mult = mybir.AluOpType.mult
    add = mybir.AluOpType.add

    def cs(c):
        return slice(c * L, (c + 1) * L)

    with (
        nc.sbuf_tensor("xt0", [P, t_len, 2], f32) as xt0,
        nc.sbuf_tensor("xt1", [P, t_len, 2], f32) as xt1,
        nc.sbuf_tensor("xt2", [P, t_len, 2], f32) as xt2,
        nc.sbuf_tensor("bneg0", [P, L], f32) as bneg0,
        nc.sbuf_tensor("bneg1", [P, L], f32) as bneg1,
        nc.sbuf_tensor("e40", [P, L], f32) as e40,
        nc.sbuf_tensor("e41", [P, L], f32) as e41,
        nc.sbuf_tensor("sig0", [P, t_len], f32) as sig0,
        nc.sbuf_tensor("sig1", [P, t_len], f32) as sig1,
        nc.psum_tensor("pa0", [P, L], f32) as pa0,
        nc.psum_tensor("pa1", [P, L], f32) as pa1,
        nc.psum_tensor("pg0", [P, L], f32) as pg0,
        nc.psum_tensor("pg1", [P, L], f32) as pg1,
        nc.semaphore("act_a") as act_a,        # +1 per a chunk (ACT)
        nc.semaphore("act_e") as act_e,        # +1 per e4 chunk (ACT)
        nc.semaphore("pool_seq") as pool_seq,  # +1 per POOL instruction
        nc.semaphore("dve_seq") as dve_seq,    # +1 per DVE instruction
        nc.Block(no_gpsimd_drain=True) as block,
    ):
        sem_in = [
            [nc.alloc_semaphore(f"in{s}_{c}") for c in range(CH)]
            for s in range(XT_BUFS)
        ]
        sem_out = [[nc.alloc_semaphore(f"out{s}_{c}") for c in range(CH)] for s in range(2)]
        xt = [xt0, xt1, xt2]
        bneg = [bneg0, bneg1]
        e4 = [e40, e41]
        sig = [sig0, sig1]
        pa = [pa0, pa1]
        pg = [pg0, pg1]
        # q = CH*i + c. DVE: 2 instrs per chunk (scan -> 2q+1, sigma -> 2q+2).
        # POOL: 1 instr per chunk (bneg -> q+1). ACT: 1 a per chunk (act_a -> q+1).

        @block.sync
        def _(sync):
            for i in range(n_tiles):
                for c in range(CH):
                    if i >= XT_BUFS:
                        # xt slot chunk reuse: sigma(i-XT_BUFS, c) transitively
                        # implies every reader of that chunk finished.
                        sync.wait_ge(dve_seq, 2 * (CH * (i - XT_BUFS) + c) + 2)
                    sync.dma_start(
                        xt[i % XT_BUFS][:, cs(c), :], xr[i][:, cs(c), :]
                    ).then_inc(sem_in[i % XT_BUFS][c], 16)

        @block.gpsimd
        def _(gpsimd):
            for i in range(n_tiles):
                for c in range(CH):
                    q = CH * i + c
                    dtv = xt[i % XT_BUFS][:, cs(c), 1]
                    gpsimd.wait_ge(sem_in[i % XT_BUFS][c], 16 * (i // XT_BUFS + 1))
                    gpsimd.wait_ge(act_e, q + 1)   # e4(q) ready
                    if q >= 2:
                        # bneg slot WAR: scan(q-2) was the last reader.
                        gpsimd.wait_ge(dve_seq, 2 * (q - 2) + 1)
                    # bneg = dt * (-E*K * eps)   (TensorScalarPtr is not legal
                    # on Pool, so the -E*K scale rides on ACT's e4 pass)
                    gpsimd.tensor_tensor(
                        bneg[q % 2][:], dtv, e4[q % 2][:], mult,
                    ).then_inc(pool_seq, 1)

        @block.scalar
        def _(scalar):
            def store(k):
                i, c = divmod(k, CH)
                scalar.wait_ge(dve_seq, 2 * k + 2)   # sigma(k) complete
                scalar.dma_start(
                    yr[i][:, cs(c)], sig[i % 2][:, cs(c)]
                ).then_inc(sem_out[i % 2][c], 16)

            for i in range(n_tiles):
                for c in range(CH):
                    q = CH * i + c
                    scalar.wait_ge(sem_in[i % XT_BUFS][c], 16 * (i // XT_BUFS + 1))
                    if q >= 2:
                        # a slot WAR: scan(q-2) read it.
                        scalar.wait_ge(dve_seq, 2 * (q - 2) + 1)
                    # a = Copy(dt * -K + 1) -> PSUM
                    scalar.activation(
                        pa[q % 2][:], xt[i % XT_BUFS][:, cs(c), 1],
                        mybir.ActivationFunctionType.Copy,
                        bias=1.0, scale=-K,
                    ).then_inc(act_a, 1)
                    if q >= 2:
                        # e4 slot WAR: bneg(q-2) read it.
                        scalar.wait_ge(pool_seq, q - 1)
                    # e4 = Copy(eps * -E*K) -> SBUF (feeds POOL's bneg)
                    scalar.activation(
                        e4[q % 2][:], xt[i % XT_BUFS][:, cs(c), 0],
                        mybir.ActivationFunctionType.Copy,
                        bias=0.0, scale=NEG_EK,
                    ).then_inc(act_e, 1)
                    if q >= 1:
                        store(q - 1)
            store(CH * n_tiles - 1)
            for c in range(CH):
                scalar.wait_ge(sem_out[0][c], 16 * ((n_tiles + 1) // 2))
                if n_tiles >= 2:
                    scalar.wait_ge(sem_out[1][c], 16 * (n_tiles // 2))

        @block.vector
        def _(vector):
            for i in range(n_tiles):
                for c in range(CH):
                    q = CH * i + c
                    eps = xt[i % XT_BUFS][:, cs(c), 0]
                    vector.wait_ge(sem_in[i % XT_BUFS][c], 16 * (i // XT_BUFS + 1))
                    vector.wait_ge(act_a, q + 1)       # a(q) in PSUM
                    vector.wait_ge(pool_seq, q + 1)    # bneg(q) in SBUF
                    if q >= 1:
                        vector.wait_ge(dve_seq, 2 * q)  # sigma(q-1) complete
                    # g_n = a_n*g_{n-1} + bneg_n  ->  g = -E*gamma
                    # Chain across chunks: initial = last element of the
                    # previous chunk's g; fresh 0 at each tile's chunk 0.
                    init = 0.0 if c == 0 else pg[(q - 1) % 2][:, L - 1:L]
                    vector.tensor_tensor_scan(
                        pg[q % 2][:], pa[q % 2][:], bneg[q % 2][:], init, mult, add,
                    ).then_inc(dve_seq, 1)
                    if i >= 2:
                        # sig slot chunk reuse: store(i-2, c) completed.
                        vector.wait_ge(sem_out[i % 2][c], 16 * ((i - 2) // 2 + 1))
                    vector.wait_ge(dve_seq, 2 * q + 1)   # scan complete
                    # sigma = (eps * 2.5) + g
                    vector.scalar_tensor_tensor(
                        sig[i % 2][:, cs(c)], eps, SIG_EPS, pg[q % 2][:], mult, add,
                    ).then_inc(dve_seq, 1)

    return nc


_NC_CACHE: dict = {}


def _get_nc(b_shard: int, t_len: int) -> bass.Bass:
    key = (b_shard, t_len)
    if key not in _NC_CACHE:
        _NC_CACHE[key] = build_nc(b_shard, t_len)
    return _NC_CACHE[key]


def run(x: np.ndarray, trace: bool = False):
    """Run the sharded kernel; returns (full_output, BassKernelResults)."""
    b, t_len, c = x.shape
    assert c == 2 and b % N_CORES == 0
    b_shard = b // N_CORES
    x = np.ascontiguousarray(np.asarray(x, dtype=np.float32))
    shards = x.reshape(N_CORES, b_shard, t_len, 2)
    in_maps = [{"x": shards[i]} for i in range(N_CORES)]
    res = run_bass_kernel_spmd(
        _get_nc(b_shard, t_len), in_maps,
        core_ids=list(range(N_CORES)), trace=trace,
    )
    out = np.concatenate([r["y"] for r in res.results], axis=0)
    return out.reshape(b, t_len, 1), res


def kernel(x: np.ndarray) -> np.ndarray:
    out, _ = run(x, trace=False)
    return out



# revision 5
# speedup vs baseline: 1.0684x; 1.0684x over previous
"""Maxwell viscoelastic model (linear recurrence scan) on 8 Trainium2 NeuronCores.

Math (per trajectory, T timesteps):
    a_n = 1 - k*dt_n                 (k = E/eta = 2)
    t_n = a_n*t_{n-1} + dt_n*eps_n   (t = gamma/k by linearity, t_0 = 0)
    sigma_n = 2.5*eps_n - 4*t_n

Strategy: batch (4096 trajectories) sharded across 8 cores (512 each).
All HBM traffic in bf16 (tolerance is 2e-2; measured pipeline error ~1%):
host deinterleaves x[:, :, 2] into eps/dt planes so every on-chip operand
is a dense step-1 bf16 vector.  Per core, 4 tiles of [128 x 4096], cut in
1024-step chunks that stream through a 5-stage software pipeline:

  SYNC  whole-tile 1 MB dt/eps loads (ring of 3 tiles, qSPDynamicHW)
  ACT   a = 1 - 2*dt -> PSUM f32 (PSUM data0 keeps the DVE scan off the
        SBUF read port GpSimd shares), and sigma PSUM->SBUF bf16 copies,
        and the output stores (qActDynamicHW)
  POOL  p = dt*eps -> SBUF bf16 (clean 2ns/elem; runs beside the scan
        because the scan's only SBUF read is data1)
  DVE   t = scan(a[PSUM], p[SBUF]) -> SBUF bf16 (2 cyc/elem, the hard
        floor: TensorScalarPtr ops are DVE-only and scan has no 2x mode)
  PE    sigma = -4*t + 2.5*eps as two diagonal-weight matmuls
        accumulated in PSUM f32 (PE is otherwise idle; diag(-4)/diag(2.5)
        ship from host as constant weights)

Raw bass (no TileContext), one semaphore per producer stage; every
cross-engine and same-engine RAW goes through a then_inc completion
counter because engine pipelines ack SBUF writes after the next
instruction may issue.  PSUM is exactly full: a-chunks 2x4KB double
buffered + sigma-chunks 2x4KB.
"""

from contextlib import ExitStack

import numpy as np
import ml_dtypes

import concourse.bass as bass
import concourse.mybir as mybir
from concourse.bass_utils import run_bass_kernel_spmd

N_CORES = 8
P = 128                      # SBUF partitions
T_LEN = 4096                 # timesteps per trajectory
B_SHARD = 512                # trajectories per core
N_TILES = B_SHARD // P       # 4
CPT = 4                      # chunks per tile
CL = T_LEN // CPT            # 1024 chunk length
NQ = N_TILES * CPT           # 16 chunks per core
XT_BUFS = 3                  # input tile ring depth
MM = 512                     # matmul moving-free max

BF16 = ml_dtypes.bfloat16


def build_nc() -> bass.Bass:
    nc = bass.Bass()
    f32 = mybir.dt.float32
    bf16 = mybir.dt.bfloat16
    mult = mybir.AluOpType.mult
    add = mybir.AluOpType.add
    Copy = mybir.ActivationFunctionType.Copy

    dt_d = nc.dram_tensor("dt", [B_SHARD, T_LEN], bf16, kind="ExternalInput")
    eps_d = nc.dram_tensor("eps", [B_SHARD, T_LEN], bf16, kind="ExternalInput")
    w4_d = nc.dram_tensor("w4", [P, P], bf16, kind="ExternalInput")
    w25_d = nc.dram_tensor("w25", [P, P], bf16, kind="ExternalInput")
    y_d = nc.dram_tensor("y", [B_SHARD, T_LEN], bf16, kind="ExternalOutput")

    dtr = dt_d.rearrange("(n p) t -> n p t", p=P)    # [4, 128, 4096]
    epr = eps_d.rearrange("(n p) t -> n p t", p=P)
    yr = y_d.rearrange("(n p) t -> n p t", p=P)

    def cs(c):
        return slice(c * CL, (c + 1) * CL)

    with ExitStack() as st:
        ec = st.enter_context
        dt_t = [ec(nc.sbuf_tensor(f"dt{s}", [P, T_LEN], bf16)) for s in range(XT_BUFS)]
        ep_t = [ec(nc.sbuf_tensor(f"ep{s}", [P, T_LEN], bf16)) for s in range(XT_BUFS)]
        t_t = [ec(nc.sbuf_tensor(f"t{s}", [P, CL], bf16)) for s in range(2)]
        p_t = [ec(nc.sbuf_tensor(f"p{s}", [P, CL], bf16)) for s in range(2)]
        sig = [ec(nc.sbuf_tensor(f"sig{s}", [P, T_LEN], bf16)) for s in range(2)]
        sw4 = ec(nc.sbuf_tensor("sw4", [P, P], bf16))
        sw25 = ec(nc.sbuf_tensor("sw25", [P, P], bf16))
        pa = [ec(nc.psum_tensor(f"pa{s}", [P, CL], f32)) for s in range(2)]
        ps = [ec(nc.psum_tensor(f"ps{s}", [P, CL], f32)) for s in range(2)]
        block = ec(nc.Block(no_gpsimd_drain=True))

        sem_x = [nc.alloc_semaphore(f"x{s}") for s in range(XT_BUFS)]
        sem_w = nc.alloc_semaphore("w")
        act_a = nc.alloc_semaphore("act_a")    # +1 per a chunk
        act_cp = nc.alloc_semaphore("act_cp")  # +1 per sigma copy PAIR (2 chunks)
        gps_p = nc.alloc_semaphore("gps_p")    # +1 per p chunk
        dve_s = nc.alloc_semaphore("dve_s")    # +1 per scan chunk
        pe_g = nc.alloc_semaphore("pe_g")      # +1 per matmul group (2 per chunk)
        sem_out = [nc.alloc_semaphore(f"out{s}") for s in range(2)]

        # psig pair view: ps0|ps1 are adjacent PSUM -> treat copies per pair
        # explicitly (ps[0] then ps[1] in one AP is not contiguous as tensors,
        # so the pair copy reads each separately; see ACT below).

        @block.sync
        def _(sync):
            sync.dma_start(sw4[:], w4_d[:, :]).then_inc(sem_w, 16)
            sync.dma_start(sw25[:], w25_d[:, :]).then_inc(sem_w, 16)
            for i in range(N_TILES):
                if i >= XT_BUFS:
                    # slot reuse: all readers of tile i-XT_BUFS finished
                    j = i - XT_BUFS
                    sync.wait_ge(act_a, CPT * (j + 1))
                    sync.wait_ge(gps_p, CPT * (j + 1))
                    sync.wait_ge(pe_g, 2 * CPT * (j + 1))
                sync.dma_start(dt_t[i % XT_BUFS][:, :], dtr[i][:, :]).then_inc(
                    sem_x[i % XT_BUFS], 16)
                sync.dma_start(ep_t[i % XT_BUFS][:, :], epr[i][:, :]).then_inc(
                    sem_x[i % XT_BUFS], 16)

        @block.gpsimd
        def _(gpsimd):
            for i in range(N_TILES):
                for c in range(CPT):
                    q = CPT * i + c
                    gpsimd.wait_ge(sem_x[i % XT_BUFS], 32 * (i // XT_BUFS + 1))
                    if q >= 2:
                        # p slot WAR: scan(q-2) was the reader
                        gpsimd.wait_ge(dve_s, q - 1)
                    gpsimd.tensor_tensor(
                        p_t[q % 2][:], dt_t[i % XT_BUFS][:, cs(c)],
                        ep_t[i % XT_BUFS][:, cs(c)], mult,
                    ).then_inc(gps_p, 1)

        @block.vector
        def _(vector):
            for i in range(N_TILES):
                for c in range(CPT):
                    q = CPT * i + c
                    vector.wait_ge(act_a, q + 1)
                    vector.wait_ge(gps_p, q + 1)
                    if q >= 1:
                        # scan(q-1) complete: init RAW (c>0) or t-slot
                        # overwrite vs its init read (c==0)
                        vector.wait_ge(dve_s, q)
                    if q >= 2:
                        # t slot WAR: PE groups of chunk q-2 done
                        vector.wait_ge(pe_g, 2 * (q - 1))
                    init = 0.0 if c == 0 else t_t[(q - 1) % 2][:, CL - 1:CL]
                    vector.tensor_tensor_scan(
                        t_t[q % 2][:], pa[q % 2][:], p_t[q % 2][:], init,
                        mult, add,
                    ).then_inc(dve_s, 1)

        @block.tensor
        def _(pe):
            pe.wait_ge(sem_w, 32)
            for i in range(N_TILES):
                for c in range(CPT):
                    q = CPT * i + c
                    if c == 0:
                        pe.wait_ge(sem_x[i % XT_BUFS], 32 * (i // XT_BUFS + 1))
                    pe.wait_ge(dve_s, q + 1)       # t(q) ready
                    if q >= 2:
                        # sigma slot WAR: copy of chunk q-2 done
                        pe.wait_ge(act_cp, q - 1)
                    for s in range(2):
                        sub = slice(s * MM, (s + 1) * MM)
                        esub = slice(c * CL + s * MM, c * CL + (s + 1) * MM)
                        pe.matmul(ps[q % 2][:, sub], sw4[:], t_t[q % 2][:, sub],
                                  start=True, stop=False)
                        pe.matmul(ps[q % 2][:, sub], sw25[:],
                                  ep_t[i % XT_BUFS][:, esub],
                                  start=False, stop=True).then_inc(pe_g, 1)

        @block.scalar
        def _(scalar):
            def copy_pair(j):
                # sigma copies for chunks 2j (ps0) and 2j+1 (ps1)
                q0 = 2 * j
                i0, c0 = divmod(q0, CPT)
                scalar.wait_ge(pe_g, 2 * (q0 + 2))   # both chunks' groups done
                if i0 >= 2:
                    scalar.wait_ge(sem_out[i0 % 2], 16 * ((i0 - 2) // 2 + 1))
                scalar.activation(sig[i0 % 2][:, cs(c0)], ps[0][:], Copy
                                  ).then_inc(act_cp, 1)
                scalar.activation(sig[i0 % 2][:, cs(c0 + 1)], ps[1][:], Copy
                                  ).then_inc(act_cp, 1)
                if c0 + 1 == CPT - 1:
                    # tile i0 fully copied -> store it (act_cp counts copies)
                    scalar.wait_ge(act_cp, q0 + 2)
                    scalar.dma_start(yr[i0][:, :], sig[i0 % 2][:, :]).then_inc(
                        sem_out[i0 % 2], 16)

            for step in range(NQ + 2):
                if step < NQ:
                    q = step
                    i, c = divmod(q, CPT)
                    scalar.wait_ge(sem_x[i % XT_BUFS], 32 * (i // XT_BUFS + 1))
                    if q >= 2:
                        # pa slot WAR: scan(q-2) read it
                        scalar.wait_ge(dve_s, q - 1)
                    scalar.activation(pa[q % 2][:], dt_t[i % XT_BUFS][:, cs(c)],
                                      Copy, bias=1.0, scale=-2.0
                                      ).then_inc(act_a, 1)
                j2 = step - 2
                if j2 >= 0 and j2 % 2 == 0:
                    copy_pair(j2 // 2)
            for s2 in range(2):
                scalar.wait_ge(sem_out[s2], 16 * 2)

    return nc


_NC_CACHE: dict = {}


def _get_nc() -> bass.Bass:
    if "nc" not in _NC_CACHE:
        _NC_CACHE["nc"] = build_nc()
    return _NC_CACHE["nc"]


def run(x: np.ndarray, trace: bool = False):
    """Run the sharded kernel; returns (full_output, BassKernelResults)."""
    b, t_len, ch = x.shape
    assert ch == 2 and b == N_CORES * B_SHARD and t_len == T_LEN
    x = np.asarray(x, dtype=np.float32)
    eps = np.ascontiguousarray(x[:, :, 0]).astype(BF16)
    dt = np.ascontiguousarray(x[:, :, 1]).astype(BF16)
    w4 = (np.eye(P, dtype=np.float32) * -4.0).astype(BF16)
    w25 = (np.eye(P, dtype=np.float32) * 2.5).astype(BF16)
    eps_sh = eps.reshape(N_CORES, B_SHARD, T_LEN)
    dt_sh = dt.reshape(N_CORES, B_SHARD, T_LEN)
    in_maps = [
        {"dt": dt_sh[i], "eps": eps_sh[i], "w4": w4, "w25": w25}
        for i in range(N_CORES)
    ]
    res = run_bass_kernel_spmd(
        _get_nc(), in_maps, core_ids=list(range(N_CORES)), trace=trace,
    )
    out = np.concatenate([r["y"].astype(np.float32) for r in res.results], axis=0)
    return out.reshape(b, t_len, 1), res


def kernel(x: np.ndarray) -> np.ndarray:
    out, _ = run(x, trace=False)
    return out


# revision 11
# speedup vs baseline: 1.4025x; 1.3128x over previous
"""Maxwell viscoelastic model (linear recurrence scan) on 8 Trainium2 NeuronCores.

Math (per trajectory, T timesteps):
    a_n = 1 - k*dt_n                 (k = E/eta = 2)
    t_n = a_n*t_{n-1} + dt_n*eps_n   (t = gamma/k by linearity, t_0 = 0)
    sigma_n = 2.5*eps_n - 4*t_n

Strategy: batch (4096 trajectories) sharded across 8 cores (512 each).
All HBM traffic in bf16 (tolerance is 2e-2; measured pipeline error ~1%):
host deinterleaves x[:, :, 2] into eps/dt planes so every on-chip operand
is a dense step-1 bf16 vector.  Per core, 4 tiles of [128 x 4096], cut in
1024-step chunks that stream through a software pipeline:

  SYNC  dt/eps loads (first tile chunked so compute starts early, then
        whole-tile 1 MB transfers; ring of 3 tiles, qSPDynamicHW)
  ACT   a = 1 - 2*dt -> PSUM f32 (PSUM data0 keeps the DVE scan off the
        SBUF read port GpSimd shares — measured: scan(SBUF,SBUF)
        running beside a GpSimd op halves BOTH), one merged [128,2048]
        sigma PSUM->SBUF bf16 copy per chunk pair, output stores
  POOL  p = dt*eps -> SBUF bf16 (~2ns/elem, clean beside the scan)
  DVE   t = scan(a[PSUM], p[SBUF]) -> SBUF bf16: 2 cyc/elem serial
        feedback, the hard floor; DVE does nothing else
  PE    sigma = -4*t + 2.5*eps: per chunk ldw(W4), mm halves -> PSUM
        start, ldw(W25), mm halves accumulate (diag weights from host)

The scheduling trap this layout dodges: ACT executes in order, so a
sigma-copy that waits on a *recent* PE result would also block the next
a-pass and serialize the whole ring (scan->PE->copy->a->scan).  Copies
are emitted one chunk later than their inputs strictly allow, so every
a-pass the scan needs is issued before ACT blocks on PE.

Raw bass; every cross-engine and same-engine RAW goes through then_inc
completion counters (engine pipelines ack writes late).  PSUM exactly
full: a-chunks 2x4KB + sigma pair buffer 8KB.
"""

from contextlib import ExitStack

import numpy as np
import ml_dtypes

import concourse.bass as bass
import concourse.mybir as mybir
from concourse.bass_utils import run_bass_kernel_spmd

N_CORES = 8
P = 128                      # SBUF partitions
T_LEN = 4096                 # timesteps per trajectory
B_SHARD = 512                # trajectories per core
N_TILES = B_SHARD // P       # 4
CPT = 4                      # chunks per tile
CL = T_LEN // CPT            # 1024 chunk length
NQ = N_TILES * CPT           # 16 chunks per core
XT_BUFS = 3                  # input tile ring depth
TP_BUFS = 3                  # t/p slot ring depth
MM = 512                     # matmul moving-free max

BF16 = ml_dtypes.bfloat16


def build_nc() -> bass.Bass:
    nc = bass.Bass()
    f32 = mybir.dt.float32
    bf16 = mybir.dt.bfloat16
    mult = mybir.AluOpType.mult
    add = mybir.AluOpType.add
    Copy = mybir.ActivationFunctionType.Copy

    dt_d = nc.dram_tensor("dt", [B_SHARD, T_LEN], bf16, kind="ExternalInput")
    eps_d = nc.dram_tensor("eps", [B_SHARD, T_LEN], bf16, kind="ExternalInput")
    w4_d = nc.dram_tensor("w4", [P, P], bf16, kind="ExternalInput")
    w25_d = nc.dram_tensor("w25", [P, P], bf16, kind="ExternalInput")
    y_d = nc.dram_tensor("y", [B_SHARD, T_LEN], bf16, kind="ExternalOutput")

    dtr = dt_d.rearrange("(n p) t -> n p t", p=P)    # [4, 128, 4096]
    epr = eps_d.rearrange("(n p) t -> n p t", p=P)
    yr = y_d.rearrange("(n p) t -> n p t", p=P)

    def cs(c):
        return slice(c * CL, (c + 1) * CL)

    with ExitStack() as st:
        ec = st.enter_context
        dt_t = [ec(nc.sbuf_tensor(f"dt{s}", [P, T_LEN], bf16)) for s in range(XT_BUFS)]
        ep_t = [ec(nc.sbuf_tensor(f"ep{s}", [P, T_LEN], bf16)) for s in range(XT_BUFS)]
        t_t = [ec(nc.sbuf_tensor(f"t{s}", [P, CL], bf16)) for s in range(TP_BUFS)]
        p_t = [ec(nc.sbuf_tensor(f"p{s}", [P, CL], bf16)) for s in range(TP_BUFS)]
        sig = [ec(nc.sbuf_tensor(f"sig{s}", [P, T_LEN], bf16)) for s in range(2)]
        sw4 = ec(nc.sbuf_tensor("sw4", [P, P], bf16))
        sw25 = ec(nc.sbuf_tensor("sw25", [P, P], bf16))
        pa = [ec(nc.psum_tensor(f"pa{s}", [P, CL], f32)) for s in range(2)]
        psig = ec(nc.psum_tensor("psig", [P, 2 * CL], f32))  # sigma pair buffer
        block = ec(nc.Block(no_gpsimd_drain=True))

        sem_x = [nc.alloc_semaphore(f"x{s}") for s in range(XT_BUFS)]
        sem_x0c = [nc.alloc_semaphore(f"x0c{c}") for c in range(CPT)]
        sem_w = nc.alloc_semaphore("w")
        act_a = nc.alloc_semaphore("act_a")    # +1 per a chunk
        act_cp = nc.alloc_semaphore("act_cp")  # +1 per sigma PAIR copy
        gps_p = nc.alloc_semaphore("gps_p")    # +1 per p chunk
        dve_s = nc.alloc_semaphore("dve_s")    # +1 per scan chunk
        pe_g = nc.alloc_semaphore("pe_g")      # +2 per chunk (mm half-groups)
        sem_out = [nc.alloc_semaphore(f"out{s}") for s in range(2)]

        # tile0 is DMA'd chunk by chunk, each chunk with its own semaphore
        # (completions on one queue can reorder, so one counter can't tell
        # which chunk landed); bulk tiles count 32 on their slot semaphore.
        def x_wait(eng, i, c):
            if i == 0:
                eng.wait_ge(sem_x0c[c], 32)
            else:
                eng.wait_ge(sem_x[i % XT_BUFS], 32)

        @block.sync
        def _(sync):
            # tile 0 chunk-interleaved so the pipeline fills fast
            for c in range(CPT):
                sync.dma_start(dt_t[0][:, cs(c)], dtr[0][:, cs(c)]).then_inc(
                    sem_x0c[c], 16)
                sync.dma_start(ep_t[0][:, cs(c)], epr[0][:, cs(c)]).then_inc(
                    sem_x0c[c], 16)
                if c == 0:
                    sync.dma_start(sw4[:], w4_d[:, :]).then_inc(sem_w, 16)
                    sync.dma_start(sw25[:], w25_d[:, :]).then_inc(sem_w, 16)
            for i in range(1, N_TILES):
                if i >= XT_BUFS:
                    j = i - XT_BUFS
                    sync.wait_ge(act_a, CPT * (j + 1))
                    sync.wait_ge(gps_p, CPT * (j + 1))
                    sync.wait_ge(pe_g, 2 * CPT * (j + 1))
                sync.dma_start(dt_t[i % XT_BUFS][:, :], dtr[i][:, :]).then_inc(
                    sem_x[i % XT_BUFS], 16)
                sync.dma_start(ep_t[i % XT_BUFS][:, :], epr[i][:, :]).then_inc(
                    sem_x[i % XT_BUFS], 16)

        @block.gpsimd
        def _(gpsimd):
            for i in range(N_TILES):
                for c in range(CPT):
                    q = CPT * i + c
                    x_wait(gpsimd, i, c)
                    if q >= TP_BUFS:
                        # p slot WAR: scan(q-TP_BUFS) was the reader
                        gpsimd.wait_ge(dve_s, q - TP_BUFS + 1)
                    gpsimd.tensor_tensor(
                        p_t[q % TP_BUFS][:], dt_t[i % XT_BUFS][:, cs(c)],
                        ep_t[i % XT_BUFS][:, cs(c)], mult,
                    ).then_inc(gps_p, 1)

        @block.vector
        def _(vector):
            for i in range(N_TILES):
                for c in range(CPT):
                    q = CPT * i + c
                    vector.wait_ge(act_a, q + 1)
                    vector.wait_ge(gps_p, q + 1)
                    if q >= 1:
                        # scan(q-1) complete: init RAW / t-slot WAR vs init
                        vector.wait_ge(dve_s, q)
                    if q >= TP_BUFS:
                        # t slot WAR: PE half-groups of chunk q-TP_BUFS done
                        vector.wait_ge(pe_g, 2 * (q - TP_BUFS + 1))
                    init = 0.0 if c == 0 else t_t[(q - 1) % TP_BUFS][:, CL - 1:CL]
                    vector.tensor_tensor_scan(
                        t_t[q % TP_BUFS][:], pa[q % 2][:], p_t[q % TP_BUFS][:],
                        init, mult, add,
                    ).then_inc(dve_s, 1)

        @block.tensor
        def _(pe):
            pe.wait_ge(sem_w, 32)
            for i in range(N_TILES):
                for c in range(CPT):
                    q = CPT * i + c
                    half = (q % 2) * CL
                    x_wait(pe, i, c)
                    pe.wait_ge(dve_s, q + 1)       # t(q) ready
                    if q >= 2:
                        # sigma half WAR: pair copy covering chunk q-2 done
                        pe.wait_ge(act_cp, (q - 2) // 2 + 1)
                    # one ldweights per weight: W4 over both halves (PSUM
                    # start), then W25 accumulating both halves
                    for s in range(2):
                        sub = slice(half + s * MM, half + (s + 1) * MM)
                        tsub = slice(s * MM, (s + 1) * MM)
                        pe.matmul(psig[:, sub], sw4[:], t_t[q % TP_BUFS][:, tsub],
                                  start=True, stop=False)
                    for s in range(2):
                        sub = slice(half + s * MM, half + (s + 1) * MM)
                        esub = slice(c * CL + s * MM, c * CL + (s + 1) * MM)
                        pe.matmul(psig[:, sub], sw25[:],
                                  ep_t[i % XT_BUFS][:, esub],
                                  start=False, stop=True).then_inc(pe_g, 1)

        @block.scalar
        def _(scalar):
            def copy_pair(j):
                # merged sigma copy for chunks 2j, 2j+1 (one [P, 2048] pass)
                q0 = 2 * j
                i0, c0 = divmod(q0, CPT)
                scalar.wait_ge(pe_g, 2 * (q0 + 2))   # both chunks' groups done
                if i0 >= 2:
                    scalar.wait_ge(sem_out[i0 % 2], 16 * ((i0 - 2) // 2 + 1))
                scalar.activation(
                    sig[i0 % 2][:, c0 * CL:(c0 + 2) * CL], psig[:, :], Copy,
                ).then_inc(act_cp, 1)
                if i0 < N_TILES - 1:
                    if c0 + 1 == CPT - 1:
                        # tile fully copied -> whole-tile store
                        scalar.wait_ge(act_cp, j + 1)
                        scalar.dma_start(yr[i0][:, :], sig[i0 % 2][:, :]
                                         ).then_inc(sem_out[i0 % 2], 16)
                else:
                    # last tile: store per pair to shorten the drain
                    scalar.wait_ge(act_cp, j + 1)
                    scalar.dma_start(yr[i0][:, c0 * CL:(c0 + 2) * CL],
                                     sig[i0 % 2][:, c0 * CL:(c0 + 2) * CL]
                                     ).then_inc(sem_out[i0 % 2], 16)

            for step in range(NQ + 3):
                if step < NQ:
                    q = step
                    i, c = divmod(q, CPT)
                    x_wait(scalar, i, c)
                    if q >= 2:
                        # pa slot WAR: scan(q-2) read it
                        scalar.wait_ge(dve_s, q - 1)
                    scalar.activation(pa[q % 2][:], dt_t[i % XT_BUFS][:, cs(c)],
                                      Copy, bias=1.0, scale=-2.0
                                      ).then_inc(act_a, 1)
                # copy pair j fires at step 2j+3 (one step later than its
                # inputs allow, keeping a-passes ahead of the PE wait)
                if step >= 3 and (step - 3) % 2 == 0:
                    copy_pair((step - 3) // 2)
            scalar.wait_ge(sem_out[0], 16 * 2)
            # slot1 stores: tile 1 whole + last-tile pairs (16 + 2*16)
            scalar.wait_ge(sem_out[1], 16 * 3)

    return nc


_NC_CACHE: dict = {}


def _get_nc() -> bass.Bass:
    if "nc" not in _NC_CACHE:
        _NC_CACHE["nc"] = build_nc()
    return _NC_CACHE["nc"]


def run(x: np.ndarray, trace: bool = False):
    """Run the sharded kernel; returns (full_output, BassKernelResults)."""
    b, t_len, ch = x.shape
    assert ch == 2 and b == N_CORES * B_SHARD and t_len == T_LEN
    x = np.asarray(x, dtype=np.float32)
    eps = np.ascontiguousarray(x[:, :, 0]).astype(BF16)
    dt = np.ascontiguousarray(x[:, :, 1]).astype(BF16)
    w4 = (np.eye(P, dtype=np.float32) * -4.0).astype(BF16)
    w25 = (np.eye(P, dtype=np.float32) * 2.5).astype(BF16)
    eps_sh = eps.reshape(N_CORES, B_SHARD, T_LEN)
    dt_sh = dt.reshape(N_CORES, B_SHARD, T_LEN)
    in_maps = [
        {"dt": dt_sh[i], "eps": eps_sh[i], "w4": w4, "w25": w25}
        for i in range(N_CORES)
    ]
    res = run_bass_kernel_spmd(
        _get_nc(), in_maps, core_ids=list(range(N_CORES)), trace=trace,
    )
    out = np.concatenate([r["y"].astype(np.float32) for r in res.results], axis=0)
    return out.reshape(b, t_len, 1), res


def kernel(x: np.ndarray) -> np.ndarray:
    out, _ = run(x, trace=False)
    return out


# revision 13
# speedup vs baseline: 1.4526x; 1.0357x over previous
"""Maxwell viscoelastic model (linear recurrence scan) on 8 Trainium2 NeuronCores.

Math (per trajectory, T timesteps):
    a_n = 1 - k*dt_n                 (k = E/eta = 2)
    t_n = a_n*t_{n-1} + dt_n*eps_n   (t = gamma/k by linearity, t_0 = 0)
    sigma_n = 2.5*eps_n - 4*t_n

Strategy: batch (4096 trajectories) sharded across 8 cores (512 each).
All HBM traffic in bf16 (tolerance is 2e-2; measured pipeline error ~1%):
host deinterleaves x[:, :, 2] into eps/dt planes so every on-chip operand
is a dense step-1 bf16 vector.  Per core, 4 tiles of [128 x 4096], cut in
1024-step chunks that stream through a software pipeline:

  SYNC  dt/eps loads (first tile chunked so compute starts early, then
        whole-tile 1 MB transfers; ring of 3 tiles, qSPDynamicHW)
  ACT   a = 1 - 2*dt -> PSUM f32 (PSUM data0 keeps the DVE scan off the
        SBUF read port GpSimd shares — measured: scan(SBUF,SBUF)
        running beside a GpSimd op halves BOTH), one merged [128,2048]
        sigma PSUM->SBUF bf16 copy per chunk pair, output stores
  POOL  p = dt*eps -> SBUF bf16 (~2ns/elem, clean beside the scan)
  DVE   t = scan(a[PSUM], p[SBUF]) -> SBUF bf16: 2 cyc/elem serial
        feedback, the hard floor; DVE does nothing else
  PE    sigma = -4*t + 2.5*eps: per chunk ldw(W4), mm halves -> PSUM
        start, ldw(W25), mm halves accumulate (diag weights from host)

The scheduling trap this layout dodges: ACT executes in order, so a
sigma-copy that waits on a *recent* PE result would also block the next
a-pass and serialize the whole ring (scan->PE->copy->a->scan).  Copies
are emitted one chunk later than their inputs strictly allow, so every
a-pass the scan needs is issued before ACT blocks on PE.

Raw bass; every cross-engine and same-engine RAW goes through then_inc
completion counters (engine pipelines ack writes late).  PSUM exactly
full: a-chunks 2x4KB + sigma pair buffer 8KB.
"""

from contextlib import ExitStack

import numpy as np
import ml_dtypes

import concourse.bass as bass
import concourse.mybir as mybir
from concourse.bass_utils import run_bass_kernel_spmd

N_CORES = 8
P = 128                      # SBUF partitions
T_LEN = 4096                 # timesteps per trajectory
B_SHARD = 512                # trajectories per core
N_TILES = B_SHARD // P       # 4
CPT = 4                      # chunks per tile
CL = T_LEN // CPT            # 1024 chunk length
NQ = N_TILES * CPT           # 16 chunks per core
XT_BUFS = 3                  # input tile ring depth
TP_BUFS = 3                  # t/p slot ring depth
MM = 512                     # matmul moving-free max

BF16 = ml_dtypes.bfloat16


def build_nc() -> bass.Bass:
    nc = bass.Bass()
    f32 = mybir.dt.float32
    bf16 = mybir.dt.bfloat16
    mult = mybir.AluOpType.mult
    add = mybir.AluOpType.add
    Copy = mybir.ActivationFunctionType.Copy

    dt_d = nc.dram_tensor("dt", [B_SHARD, T_LEN], bf16, kind="ExternalInput")
    eps_d = nc.dram_tensor("eps", [B_SHARD, T_LEN], bf16, kind="ExternalInput")
    w4_d = nc.dram_tensor("w4", [P, P], bf16, kind="ExternalInput")
    w25_d = nc.dram_tensor("w25", [P, P], bf16, kind="ExternalInput")
    y_d = nc.dram_tensor("y", [B_SHARD, T_LEN], bf16, kind="ExternalOutput")

    dtr = dt_d.rearrange("(n p) t -> n p t", p=P)    # [4, 128, 4096]
    epr = eps_d.rearrange("(n p) t -> n p t", p=P)
    yr = y_d.rearrange("(n p) t -> n p t", p=P)

    def cs(c):
        return slice(c * CL, (c + 1) * CL)

    with ExitStack() as st:
        ec = st.enter_context
        dt_t = [ec(nc.sbuf_tensor(f"dt{s}", [P, T_LEN], bf16)) for s in range(XT_BUFS)]
        ep_t = [ec(nc.sbuf_tensor(f"ep{s}", [P, T_LEN], bf16)) for s in range(XT_BUFS)]
        t_t = [ec(nc.sbuf_tensor(f"t{s}", [P, CL], bf16)) for s in range(TP_BUFS)]
        p_t = [ec(nc.sbuf_tensor(f"p{s}", [P, CL], bf16)) for s in range(TP_BUFS)]
        sig = [ec(nc.sbuf_tensor(f"sig{s}", [P, T_LEN], bf16)) for s in range(2)]
        sw4 = ec(nc.sbuf_tensor("sw4", [P, P], bf16))
        sw25 = ec(nc.sbuf_tensor("sw25", [P, P], bf16))
        pa = [ec(nc.psum_tensor(f"pa{s}", [P, CL], f32)) for s in range(2)]
        psig = ec(nc.psum_tensor("psig", [P, 2 * CL], f32))  # sigma pair buffer
        block = ec(nc.Block(no_gpsimd_drain=True))

        sem_xc = [[nc.alloc_semaphore(f"x{s}c{c}") for c in range(CPT)]
                  for s in range(XT_BUFS)]
        sem_w = nc.alloc_semaphore("w")
        act_a = nc.alloc_semaphore("act_a")    # +1 per a chunk
        act_cp = nc.alloc_semaphore("act_cp")  # +1 per sigma PAIR copy
        gps_p = nc.alloc_semaphore("gps_p")    # +1 per p chunk
        dve_s = nc.alloc_semaphore("dve_s")    # +1 per scan chunk
        pe_g = nc.alloc_semaphore("pe_g")      # +2 per chunk (mm half-groups)
        sem_out = [nc.alloc_semaphore(f"out{s}") for s in range(2)]

        # every (tile, chunk) load pair has its own semaphore: completions on
        # one queue can reorder, so a shared counter can't tell which chunk
        # landed; per-chunk gating also removes whole-tile completion cliffs.
        def x_wait(eng, i, c):
            eng.wait_ge(sem_xc[i % XT_BUFS][c], 32 * (i // XT_BUFS + 1))

        @block.sync
        def _(sync):
            for i in range(N_TILES):
                for c in range(CPT):
                    q = CPT * i + c
                    if i >= XT_BUFS:
                        # slot chunk reuse: readers of (i-XT_BUFS, c) done
                        jq = CPT * (i - XT_BUFS) + c
                        sync.wait_ge(act_a, jq + 1)
                        sync.wait_ge(gps_p, jq + 1)
                        sync.wait_ge(pe_g, 2 * (jq + 1))
                    sync.dma_start(
                        dt_t[i % XT_BUFS][:, cs(c)], dtr[i][:, cs(c)]
                    ).then_inc(sem_xc[i % XT_BUFS][c], 16)
                    sync.dma_start(
                        ep_t[i % XT_BUFS][:, cs(c)], epr[i][:, cs(c)]
                    ).then_inc(sem_xc[i % XT_BUFS][c], 16)
                    if q == 0:
                        sync.dma_start(sw4[:], w4_d[:, :]).then_inc(sem_w, 16)
                        sync.dma_start(sw25[:], w25_d[:, :]).then_inc(sem_w, 16)

        @block.gpsimd
        def _(gpsimd):
            for i in range(N_TILES):
                for c in range(CPT):
                    q = CPT * i + c
                    x_wait(gpsimd, i, c)
                    if q >= TP_BUFS:
                        # p slot WAR: scan(q-TP_BUFS) was the reader
                        gpsimd.wait_ge(dve_s, q - TP_BUFS + 1)
                    gpsimd.tensor_tensor(
                        p_t[q % TP_BUFS][:], dt_t[i % XT_BUFS][:, cs(c)],
                        ep_t[i % XT_BUFS][:, cs(c)], mult,
                    ).then_inc(gps_p, 1)

        @block.vector
        def _(vector):
            for i in range(N_TILES):
                for c in range(CPT):
                    q = CPT * i + c
                    vector.wait_ge(act_a, q + 1)
                    vector.wait_ge(gps_p, q + 1)
                    if q >= 1:
                        # scan(q-1) complete: init RAW / t-slot WAR vs init
                        vector.wait_ge(dve_s, q)
                    if q >= TP_BUFS:
                        # t slot WAR: PE half-groups of chunk q-TP_BUFS done
                        vector.wait_ge(pe_g, 2 * (q - TP_BUFS + 1))
                    init = 0.0 if c == 0 else t_t[(q - 1) % TP_BUFS][:, CL - 1:CL]
                    vector.tensor_tensor_scan(
                        t_t[q % TP_BUFS][:], pa[q % 2][:], p_t[q % TP_BUFS][:],
                        init, mult, add,
                    ).then_inc(dve_s, 1)

        @block.tensor
        def _(pe):
            pe.wait_ge(sem_w, 32)
            for i in range(N_TILES):
                for c in range(CPT):
                    q = CPT * i + c
                    half = (q % 2) * CL
                    x_wait(pe, i, c)
                    pe.wait_ge(dve_s, q + 1)       # t(q) ready
                    if q >= 2:
                        # sigma half WAR: pair copy covering chunk q-2 done
                        pe.wait_ge(act_cp, (q - 2) // 2 + 1)
                    # one ldweights per weight: W4 over both halves (PSUM
                    # start), then W25 accumulating both halves
                    for s in range(2):
                        sub = slice(half + s * MM, half + (s + 1) * MM)
                        tsub = slice(s * MM, (s + 1) * MM)
                        pe.matmul(psig[:, sub], sw4[:], t_t[q % TP_BUFS][:, tsub],
                                  start=True, stop=False)
                    for s in range(2):
                        sub = slice(half + s * MM, half + (s + 1) * MM)
                        esub = slice(c * CL + s * MM, c * CL + (s + 1) * MM)
                        pe.matmul(psig[:, sub], sw25[:],
                                  ep_t[i % XT_BUFS][:, esub],
                                  start=False, stop=True).then_inc(pe_g, 1)

        @block.scalar
        def _(scalar):
            def copy_pair(j):
                # merged sigma copy for chunks 2j, 2j+1 (one [P, 2048] pass)
                q0 = 2 * j
                i0, c0 = divmod(q0, CPT)
                scalar.wait_ge(pe_g, 2 * (q0 + 2))   # both chunks' groups done
                if i0 >= 2:
                    scalar.wait_ge(sem_out[i0 % 2], 16 * ((i0 - 2) // 2 + 1))
                scalar.activation(
                    sig[i0 % 2][:, c0 * CL:(c0 + 2) * CL], psig[:, :], Copy,
                ).then_inc(act_cp, 1)
                if i0 < N_TILES - 1:
                    if c0 + 1 == CPT - 1:
                        # tile fully copied -> whole-tile store
                        scalar.wait_ge(act_cp, j + 1)
                        scalar.dma_start(yr[i0][:, :], sig[i0 % 2][:, :]
                                         ).then_inc(sem_out[i0 % 2], 16)
                else:
                    # last tile: store per pair to shorten the drain
                    scalar.wait_ge(act_cp, j + 1)
                    scalar.dma_start(yr[i0][:, c0 * CL:(c0 + 2) * CL],
                                     sig[i0 % 2][:, c0 * CL:(c0 + 2) * CL]
                                     ).then_inc(sem_out[i0 % 2], 16)

            for step in range(NQ + 3):
                if step < NQ:
                    q = step
                    i, c = divmod(q, CPT)
                    x_wait(scalar, i, c)
                    if q >= 2:
                        # pa slot WAR: scan(q-2) read it
                        scalar.wait_ge(dve_s, q - 1)
                    scalar.activation(pa[q % 2][:], dt_t[i % XT_BUFS][:, cs(c)],
                                      Copy, bias=1.0, scale=-2.0
                                      ).then_inc(act_a, 1)
                # copy pair j fires at step 2j+3 (one step later than its
                # inputs allow, keeping a-passes ahead of the PE wait)
                if step >= 3 and (step - 3) % 2 == 0:
                    copy_pair((step - 3) // 2)
            scalar.wait_ge(sem_out[0], 16 * 2)
            # slot1 stores: tile 1 whole + last-tile pairs (16 + 2*16)
            scalar.wait_ge(sem_out[1], 16 * 3)

    return nc


_NC_CACHE: dict = {}


def _get_nc() -> bass.Bass:
    if "nc" not in _NC_CACHE:
        _NC_CACHE["nc"] = build_nc()
    return _NC_CACHE["nc"]


def run(x: np.ndarray, trace: bool = False):
    """Run the sharded kernel; returns (full_output, BassKernelResults)."""
    b, t_len, ch = x.shape
    assert ch == 2 and b == N_CORES * B_SHARD and t_len == T_LEN
    x = np.asarray(x, dtype=np.float32)
    eps = np.ascontiguousarray(x[:, :, 0]).astype(BF16)
    dt = np.ascontiguousarray(x[:, :, 1]).astype(BF16)
    w4 = (np.eye(P, dtype=np.float32) * -4.0).astype(BF16)
    w25 = (np.eye(P, dtype=np.float32) * 2.5).astype(BF16)
    eps_sh = eps.reshape(N_CORES, B_SHARD, T_LEN)
    dt_sh = dt.reshape(N_CORES, B_SHARD, T_LEN)
    in_maps = [
        {"dt": dt_sh[i], "eps": eps_sh[i], "w4": w4, "w25": w25}
        for i in range(N_CORES)
    ]
    res = run_bass_kernel_spmd(
        _get_nc(), in_maps, core_ids=list(range(N_CORES)), trace=trace,
    )
    out = np.concatenate([r["y"].astype(np.float32) for r in res.results], axis=0)
    return out.reshape(b, t_len, 1), res


def kernel(x: np.ndarray) -> np.ndarray:
    out, _ = run(x, trace=False)
    return out
